# revision 10
# baseline (speedup 1.0000x reference)
"""Multi-head causal self-attention (B=2, N=2048, D=2048, H=16) on 8 NeuronCores.

Sharding: core c handles batch b = c//4 and heads 4*(c%4) .. 4*(c%4)+3
(data parallel over batch, tensor parallel over heads).  Each core:
  - projects V first (natural [seq, head_dim] layout), then Q^T / K^T in
    [head_dim, seq] layout, contracting over d_in chunks that sit on
    partitions (x is pre-transposed on the host),
  - runs causal attention per head entirely in transposed space
    (S^T = K_tile Q^T, exp on ScalarE writes P^T straight into SBUF,
    denominators via a ones-row matmul, 1/sum fused into the ctx copy),
  - computes the partial output projection ctx_slice @ W_out[rows_slice]
    into a [2048, 2048] fp32 partial.
The host sums the 4 partials per batch and adds the output bias.

Schedule (v7, ~346 us vs the 390 us v1 baseline at the same 2.37 GHz
PE clock): input DMAs are issued in consumption order in ~0.26 MB
chunks (16 DMA engines run in parallel; one huge DMA would serialize on
a single engine at ~23 GB/s); V is projected first, then Q/K in two
n-halves, with the g=3 head-0 score block + exp woven into the tail of
the QK projection so the softmax pipeline is primed when projection
ends; scores for step s+2 are emitted as interleaved units inside step
s; exp is batched over [128,1024] PSUM pairs to amortize ACT access
latency; the causal mask is applied by zeroing the sub-diagonal
triangle of P^T on the otherwise-idle GpSimd engine after exp (exp
cannot overflow: |s*scale| <= ~9), which keeps mask work off the PE;
PSUM->SBUF out-tile copies alternate DVE/ACT in late steps; ctx and
out-proj tiles share a 3-deep PSUM ring; and out-projection tiles are
interleaved as filler work between attention steps so the PE never
idles (HW runs the PE at half clock for ~3 us after any idle).

Matmul inputs are bf16 (fp32 accumulation in PSUM); measured end-to-end
relative error vs the fp32 reference is ~5.5e-3.
"""

import math

import numpy as np
import ml_dtypes

import concourse.bass as bass
import concourse.mybir as mybir
import concourse.tile as tile
from concourse import bacc
from concourse.bass_utils import run_bass_kernel_spmd

BF16 = mybir.dt.bfloat16
F32 = mybir.dt.float32
ALU = mybir.AluOpType
ACT_EXP = mybir.ActivationFunctionType.Exp

P = 128              # partitions
D_IN = 2048          # model dim
N_SEQ = 2048         # sequence length
HD = 128             # head dim
HPC = 4              # heads per core
DC = HPC * HD        # 512: d_out slice per core
N_CORES = 8
SCALE = 1.0 / math.sqrt(HD)
NEG_BIG = -1e10

NT = N_SEQ // P      # 16 seq tiles of 128
NI = D_IN // P       # 16 contraction chunks of 128
NG = NT // 4         # 4 groups of 4 q-tiles (q-512 groups)
NJ = D_IN // 512     # 4 output column chunks


def _build_body(tc, xt_d, wq_d, wk_d, wv_d, wo_d, out_d):
    nc = tc.nc
    from contextlib import ExitStack
    ctx = ExitStack()
    with ctx:
        # ---------------- constants ----------------
        const = ctx.enter_context(tc.tile_pool(name="const", bufs=1))
        # all-ones [128,128] lhsT: the denominator matmul ones.T @ acc then
        # emits the column sums already broadcast to every partition row
        ones_sb = const.tile([P, P], BF16)
        nc.vector.memset(ones_sb, 1.0)
        warmsrc = const.tile([P, 512], BF16)
        nc.vector.memset(warmsrc, 0.0)
        # force the exp activation table load at t=0, off the critical path
        tscr = const.tile([P, 1], F32)
        nc.vector.memset(tscr, 0.0)
        nc.scalar.activation(out=tscr, in_=tscr, func=ACT_EXP, bias=0.0, scale=1.0)

        # ---------------- persistent activations ----------------
        persist = ctx.enter_context(tc.tile_pool(name="persist", bufs=1))
        qt_sb = persist.tile([P, HPC, N_SEQ], BF16)    # Q^T  [d, h, n]
        kt_sb = persist.tile([P, HPC, N_SEQ], BF16)    # K^T  [d, h, n]
        v_sb = persist.tile([P, NT, DC], BF16)         # V natural [n(128), nt, d]
        ctxT_sb = persist.tile([P, HPC, N_SEQ], BF16)  # ctx^T [d, h, n]
        # dedicated P^T tile for the score step woven into projection
        wv_ptg0 = persist.tile([P, NT, 512], BF16)
        # the denominator tree slab-adds read full [128,512] kt rows, so the
        # never-written [0:off] columns of diagonal tiles must be zeroed
        def memset_diag_holes(g, ptg):
            for kt in range(4 * g + 1, 4 * (g + 1)):
                off = (kt - 4 * g) * P
                nc.vector.memset(ptg[:, kt, 0:off], 0.0)

        memset_diag_holes(3, wv_ptg0)

        xt_r = xt_d.rearrange("(io p) nn -> p io nn", p=P)
        wq_r = wq_d.rearrange("(io p) c -> p io c", p=P)
        wk_r = wk_d.rearrange("(io p) c -> p io c", p=P)
        wv_r = wv_d.rearrange("(io p) c -> p io c", p=P)
        wo_r = wo_d.rearrange("(h p) j -> p h j", p=P)

        # step list: groups in descending size so the pipeline has big work
        # first; (g, h) pairs in emission order
        steps = [(3 - i // HPC, i % HPC) for i in range(NG * HPC)]
        LAG = 2

        # ---------------- score/exp emitter (shared proj + main) ----------
        def emit_scores_units(g, h, ptg, s_pool):
            """Yield after each PE unit: full-kt pairs then diag pairs.
            Scores for group g head h: S^T = K_kt.T @ Q^T, mask accumulated
            on the PE for diagonal blocks, batched exp into ptg (bf16)."""
            nkt = 4 * (g + 1)
            q0 = 512 * g
            nfull = 4 * g
            # full-width kt tiles, two per [128,1024] psum tensor with one
            # batched exp (amortizes ACT access latency)
            for kp in range(0, nfull, 2):
                sps = s_pool.tile([P, 1024], F32, tag="s", name="sps")
                for j, kt in enumerate((kp, kp + 1)):
                    nc.tensor.matmul(
                        sps[:, j * 512:(j + 1) * 512],
                        lhsT=kt_sb[:, h, kt * P:(kt + 1) * P],
                        rhs=qt_sb[:, h, q0:q0 + 512],
                        start=True, stop=True,
                    )
                nc.scalar.activation(
                    out=ptg[:, kp:kp + 2, :], in_=sps,
                    func=ACT_EXP, bias=0.0, scale=SCALE,
                )
                yield
            # diagonal kt tiles (off = 128*(kt-4g)), two per psum tensor;
            # exp the full block (no overflow: |s*scale| <= ~9), then zero the
            # sub-diagonal triangle of P^T on the idle GpSimd engine
            for kp in range(nfull, nkt, 2):
                sps = s_pool.tile([P, 1024], F32, tag="s", name="sps")
                for j, kt in enumerate((kp, kp + 1)):
                    off = (kt - nfull) * P
                    nc.tensor.matmul(
                        sps[:, j * 512 + off:(j + 1) * 512],
                        lhsT=kt_sb[:, h, kt * P:(kt + 1) * P],
                        rhs=qt_sb[:, h, q0 + off:q0 + 512],
                        start=True, stop=True,
                    )
                for j, kt in enumerate((kp, kp + 1)):
                    off = (kt - nfull) * P
                    nc.scalar.activation(
                        out=ptg[:, kt, off:512],
                        in_=sps[:, j * 512 + off:(j + 1) * 512],
                        func=ACT_EXP, bias=0.0, scale=SCALE,
                    )
                for j, kt in enumerate((kp, kp + 1)):
                    off = (kt - nfull) * P
                    nc.gpsimd.affine_select(
                        out=ptg[:, kt, off:off + P], in_=ptg[:, kt, off:off + P],
                        compare_op=ALU.is_ge, fill=0.0,
                        base=0, pattern=[[1, P]], channel_multiplier=-1,
                    )
                yield

        # ---------------- stage 1: projections (V first, then Q/K) -------
        with tc.tile_pool(name="xw", bufs=1) as xw_pool, \
             tc.tile_pool(name="wqk", bufs=1) as wqk_pool, \
             tc.tile_pool(name="pjps", bufs=2, space="PSUM") as pj_psum:
            xt_sb = xw_pool.tile([P, NI, N_SEQ], BF16)
            wq_sb = wqk_pool.tile([P, NI, DC], BF16)
            wk_sb = wqk_pool.tile([P, NI, DC], BF16)

            # input DMAs in consumption order, ~0.26MB chunks (2 i-rows x
            # 512 cols) so the 16 DMA engines run in parallel
            def dma_w(dst, src):
                for i0 in range(0, NI, 2):
                    nc.sync.dma_start(dst[:, i0:i0 + 2, :], src[:, i0:i0 + 2, :])

            def dma_xt_win(w, fine=False):
                c0, c1 = w * 512, (w + 1) * 512
                step = 1 if fine else 2
                for i0 in range(0, NI, step):
                    nc.sync.dma_start(xt_sb[:, i0:i0 + step, c0:c1],
                                      xt_r[:, i0:i0 + step, c0:c1])

            def emit_qk_pass(h, w_sb, dst, win):
                ps = pj_psum.tile([P, 512], F32, tag="qk")
                for i in range(NI):
                    nc.tensor.matmul(
                        ps,
                        lhsT=w_sb[:, i, h * P:(h + 1) * P],
                        rhs=xt_sb[:, i, win * 512:(win + 1) * 512],
                        start=(i == 0), stop=(i == NI - 1),
                    )
                nc.vector.tensor_copy(
                    out=dst[:, h, win * 512:(win + 1) * 512], in_=ps)

            # wv lives in an inner scope so its SBUF frees before the weave
            with tc.tile_pool(name="wvp", bufs=1) as wv_pool:
                wv_sb = wv_pool.tile([P, NI, DC], BF16)
                # first round: wv and xt-win0 i-chunks interleaved in ~0.13MB
                # units so i=0..7 of BOTH land on the first pass of all 16
                # DMA queues (~6us) and the V projection can start early
                for i in range(0, NI, 2):
                    nc.sync.dma_start(wv_sb[:, i:i + 1, :], wv_r[:, i:i + 1, :])
                    nc.sync.dma_start(xt_sb[:, i:i + 1, 0:512],
                                      xt_r[:, i:i + 1, 0:512])
                    nc.sync.dma_start(wv_sb[:, i + 1:i + 2, :],
                                      wv_r[:, i + 1:i + 2, :])
                    nc.sync.dma_start(xt_sb[:, i + 1:i + 2, 0:512],
                                      xt_r[:, i + 1:i + 2, 0:512])
                dma_xt_win(1, fine=True)
                dma_w(wq_sb, wq_r)
                dma_w(wk_sb, wk_r)
                dma_xt_win(2)
                dma_xt_win(3)

                # dummy matmuls: keep the PE busy (and ramp the HAM clock
                # gate to full speed) while the first DMAs land
                warm_ps = pj_psum.tile([P, 512], F32, tag="v", name="warm_ps")
                for _ in range(14):
                    nc.tensor.matmul(warm_ps, lhsT=warmsrc[:, :P], rhs=warmsrc,
                                     start=True, stop=True)

                def emit_v(nt):
                    ps = pj_psum.tile([P, DC], F32, tag="v")
                    for i in range(NI):
                        nc.tensor.matmul(
                            ps,
                            lhsT=xt_sb[:, i, nt * P:(nt + 1) * P],
                            rhs=wv_sb[:, i, :],
                            start=(i == 0), stop=(i == NI - 1),
                        )
                    nc.vector.tensor_copy(out=v_sb[:, nt, :], in_=ps)

                # V-A, QK-A (windows 0,1), V-B
                for nt in range(0, 8):
                    emit_v(nt)
                for h in range(HPC):
                    for w_sb, dst in ((wq_sb, qt_sb), (wk_sb, kt_sb)):
                        for win in (0, 1):
                            emit_qk_pass(h, w_sb, dst, win)
                for nt in range(8, 16):
                    emit_v(nt)

            # QK-B (windows 2,3) with the first two g=3 score steps woven in
            weave = []  # generators of woven score units

            def pump_weave(n):
                for _ in range(n):
                    for gen in list(weave):
                        try:
                            next(gen)
                            break
                        except StopIteration:
                            weave.remove(gen)

            for h in range(HPC):
                for w_sb, dst in ((wq_sb, qt_sb), (wk_sb, kt_sb)):
                    for win in (2, 3):
                        emit_qk_pass(h, w_sb, dst, win)
                        pump_weave(2)
                if h == 0:
                    weave.append(emit_scores_units(3, 0, wv_ptg0, pj_psum))
            pump_weave(100)

        # ---------------- stage 2: attention + out-projection ------------
        with tc.tile_pool(name="att", bufs=4) as att_pool, \
             tc.tile_pool(name="small", bufs=2) as small_pool, \
             tc.tile_pool(name="tsum", bufs=3) as tsum_pool, \
             tc.tile_pool(name="osb", bufs=4) as out_pool, \
             tc.tile_pool(name="sps2", bufs=2, space="PSUM") as s_pool, \
             tc.tile_pool(name="colps", bufs=1, space="PSUM") as col_pool, \
             tc.tile_pool(name="ops", bufs=3, space="PSUM") as o_pool:
            wo_sb = att_pool.tile([P, HPC, D_IN], BF16, tag="wo", bufs=1)
            for hh in range(HPC):
                for j0 in (0, 1024):
                    nc.sync.dma_start(wo_sb[:, hh, j0:j0 + 1024],
                                      wo_r[:, hh, j0:j0 + 1024])

            ptgs = {(3, 0): wv_ptg0}

            def emit_tree(g, h, ptg):
                """Pairwise bf16 slab-add tree over the nkt exp'd P^T tiles
                on the (non-critical) DVE; replaces per-kt ones-matmuls on
                the PE.  Returns the [128,512] per-partition partial sums."""
                nkt = 4 * (g + 1)
                m = nkt // 2
                sc = tsum_pool.tile([P, 8, 512], BF16, tag="ts", name="tsc")
                nc.vector.tensor_tensor(out=sc[:, 0:m, :], in0=ptg[:, 0:m, :],
                                        in1=ptg[:, m:nkt, :], op=ALU.add)
                while m > 1:
                    if m % 2 == 0:
                        m //= 2
                        nc.vector.tensor_tensor(
                            out=sc[:, 0:m, :], in0=sc[:, 0:m, :],
                            in1=sc[:, m:2 * m, :], op=ALU.add)
                    else:  # m == 3
                        nc.vector.tensor_tensor(
                            out=sc[:, 0:1, :], in0=sc[:, 0:1, :],
                            in1=sc[:, 1:2, :], op=ALU.add)
                        nc.vector.tensor_tensor(
                            out=sc[:, 0:1, :], in0=sc[:, 0:1, :],
                            in1=sc[:, 2:3, :], op=ALU.add)
                        m = 1
                return sc

            def emit_den(acc):
                """One ones.T @ acc matmul finishes the column sums over the
                partition axis, already broadcast to all 128 rows; reciprocal
                feeds the fused ctx normalize directly."""
                colp = col_pool.tile([P, 512], F32, tag="col", name="colp")
                nc.tensor.matmul(colp, lhsT=ones_sb, rhs=acc[:, 0, :],
                                 start=True, stop=True)
                recip_bc = small_pool.tile([P, 512], F32, tag="rsb",
                                           name="recip_bc")
                nc.vector.reciprocal_approx_fast(out=recip_bc, in_=colp)
                return recip_bc

            def emit_ctx(g, h, ptg, recip_bc):
                """ctx^T accumulated over k tiles, 1/colsum fused into the
                PSUM->SBUF copy."""
                nkt = 4 * (g + 1)
                cps = o_pool.tile([P, 512], F32, tag="o", name="cps")
                for kt in range(nkt):
                    off = max(kt - 4 * g, 0) * P
                    nc.tensor.matmul(
                        cps[:, off:512],
                        lhsT=v_sb[:, kt, h * P:(h + 1) * P],
                        rhs=ptg[:, kt, off:512],
                        start=(kt == 0), stop=(kt == nkt - 1),
                        skip_group_check=True,
                    )
                nc.vector.tensor_tensor(
                    out=ctxT_sb[:, h, g * 512:(g + 1) * 512],
                    in0=cps, in1=recip_bc, op=ALU.mult,
                )

            op_queue = []  # ready out-projection (nt, jc) tiles
            op_count = [0]

            def emit_op_tile(allow_act=False):
                nt, jc = op_queue.pop(0)
                ops = o_pool.tile([P, 512], F32, tag="o", name="ops")
                for hh in range(HPC):
                    nc.tensor.matmul(
                        ops,
                        lhsT=ctxT_sb[:, hh, nt * P:(nt + 1) * P],
                        rhs=wo_sb[:, hh, jc * 512:(jc + 1) * 512],
                        start=(hh == 0), stop=(hh == HPC - 1),
                    )
                osb = out_pool.tile([P, 512], BF16, tag="osb", name="osb")
                op_count[0] += 1
                if allow_act and op_count[0] % 2 == 0:
                    nc.scalar.copy(out=osb, in_=ops)
                else:
                    nc.vector.tensor_copy(out=osb, in_=ops)
                nc.sync.dma_start(
                    out_d[nt * P:(nt + 1) * P, jc * 512:(jc + 1) * 512], osb)

            next_sc = 1  # (3,0) was woven into projection
            trees = {0: emit_tree(3, 0, wv_ptg0)}
            for idx, (g, h) in enumerate(steps):
                # emit scores up to LAG steps ahead, interleaved below
                gens = []
                while next_sc <= min(idx + LAG, len(steps) - 1):
                    g2, h2 = steps[next_sc]
                    ptg2 = att_pool.tile([P, NT, 512], BF16, tag="ptg",
                                         name=f"ptg_{g2}_{h2}")
                    memset_diag_holes(g2, ptg2)
                    ptgs[(g2, h2)] = ptg2
                    gens.append(emit_scores_units(g2, h2, ptg2, s_pool))
                    next_sc += 1

                def pump(n=1):
                    while n > 0 and gens:
                        try:
                            next(gens[0])
                            n -= 1
                        except StopIteration:
                            gens.pop(0)

                budget = 4 if idx < 8 else 7
                ptg = ptgs.pop((g, h))
                recip_bc = emit_den(trees.pop(idx))
                pump(2)
                emit_ctx(g, h, ptg, recip_bc)
                # interleave remaining score units with out-proj filler
                for _ in range(10):
                    pump(1)
                    if budget > 0 and op_queue:
                        emit_op_tile(allow_act=(idx >= 6))
                        budget -= 1
                pump(100)
                while budget > 0 and op_queue:
                    emit_op_tile(allow_act=(idx >= 6))
                    budget -= 1
                # denominator tree for the NEXT step (its P^T finished during
                # this step's pumps); runs on DVE under this step's tail work
                if idx + 1 < len(steps):
                    g2, h2 = steps[idx + 1]
                    trees[idx + 1] = emit_tree(g2, h2, ptgs[(g2, h2)])
                if h == HPC - 1:
                    op_queue.extend(
                        (nt, jc) for nt in range(4 * g, 4 * g + 4)
                        for jc in range(NJ))
            while op_queue:
                emit_op_tile(allow_act=True)


def build_module():
    """Build and compile the per-core Bass module (SPMD: same program, 8 cores)."""
    nc = bacc.Bacc("TRN2", target_bir_lowering=False, debug=False,
                   num_devices=N_CORES)
    xt_d = nc.dram_tensor("xt", [D_IN, N_SEQ], BF16, kind="ExternalInput").ap()
    wq_d = nc.dram_tensor("wq", [D_IN, DC], BF16, kind="ExternalInput").ap()
    wk_d = nc.dram_tensor("wk", [D_IN, DC], BF16, kind="ExternalInput").ap()
    wv_d = nc.dram_tensor("wv", [D_IN, DC], BF16, kind="ExternalInput").ap()
    wo_d = nc.dram_tensor("wo", [DC, D_IN], BF16, kind="ExternalInput").ap()
    out_d = nc.dram_tensor("out", [N_SEQ, D_IN], BF16, kind="ExternalOutput").ap()
    with tile.TileContext(nc) as tc:
        _build_body(tc, xt_d, wq_d, wk_d, wv_d, wo_d, out_d)
    nc.compile()
    return nc


def make_in_maps(x, W_qkv, W_out):
    """Host-side sharding: per-core input dict, bf16 cast + pre-transposed x."""
    bf = ml_dtypes.bfloat16
    in_maps = []
    for c in range(N_CORES):
        b, g = divmod(c, 4)
        cs = slice(DC * g, DC * (g + 1))
        in_maps.append({
            "xt": np.ascontiguousarray(x[b].T).astype(bf),
            "wq": np.ascontiguousarray(W_qkv[:, 0 * D_IN:1 * D_IN][:, cs]).astype(bf),
            "wk": np.ascontiguousarray(W_qkv[:, 1 * D_IN:2 * D_IN][:, cs]).astype(bf),
            "wv": np.ascontiguousarray(W_qkv[:, 2 * D_IN:3 * D_IN][:, cs]).astype(bf),
            "wo": np.ascontiguousarray(W_out[cs, :]).astype(bf),
        })
    return in_maps


_NC_CACHE = {}


def get_module():
    if "nc" not in _NC_CACHE:
        _NC_CACHE["nc"] = build_module()
    return _NC_CACHE["nc"]


def run(x, W_qkv, W_out, b_out, trace=False, **trace_kwargs):
    nc = get_module()
    in_maps = make_in_maps(x, W_qkv, W_out)
    res = run_bass_kernel_spmd(nc, in_maps, core_ids=list(range(N_CORES)),
                               trace=trace, **trace_kwargs)
    parts = np.stack([np.asarray(res.results[c]["out"], dtype=np.float32)
                      for c in range(N_CORES)])
    parts = parts.reshape(2, 4, N_SEQ, D_IN)
    out = parts.sum(axis=1, dtype=np.float64).astype(np.float32)
    out += b_out.astype(np.float32)
    return out, res


def kernel(x, W_qkv, W_out, b_out):
    out, _ = run(np.asarray(x), np.asarray(W_qkv), np.asarray(W_out),
                 np.asarray(b_out))
    return out



# revision 29
# speedup vs baseline: 1.2275x; 1.2275x over previous
"""Multi-head causal self-attention (B=2, N=2048, D=2048, H=16) on 8 NeuronCores.

Sharding: core c handles batch b = c//4 and heads 4*(c%4) .. 4*(c%4)+3
(data parallel over batch, tensor parallel over heads).  Each core:
  - projects V first (natural [seq, head_dim] layout), then Q^T / K^T in
    [head_dim, seq] layout, contracting over d_in chunks that sit on
    partitions (x is pre-transposed on the host),
  - runs causal attention per head entirely in transposed space
    (S^T = K_tile Q^T, exp on ScalarE writes P^T straight into SBUF,
    denominators via a ones-row matmul, 1/sum fused into the ctx copy),
  - computes the partial output projection ctx_slice @ W_out[rows_slice]
    into a [2048, 2048] fp32 partial.
The host sums the 4 partials per batch and adds the output bias.

Schedule (v10, ~313 us vs the 390 us v1 / 347 us v7 baselines at the
same ~2.32 GHz effective PE clock): input DMAs are issued in
consumption order (first rounds as ~0.13 MB i-chunks of wv and xt-win0
interleaved across the 16 DMA queues so the 4-way-interleaved V
projection can start as soon as the first round lands); the g=3 head-0
score block + exp is woven into the tail of the QK projection; scores
for step s+2 are emitted as interleaved units inside step s; exp is
batched over [128,1024] PSUM pairs; the causal mask is applied by
zeroing the sub-diagonal triangle of P^T on the GpSimd engine after
exp (exp cannot overflow: |s*scale| <= ~9).

Softmax denominators are NOT computed with per-kt ones-row matmuls on
the PE (that costs a second full pass of P^T through the array, ~29 us
of the 128-wide PE stream at M=1): instead a pairwise bf16 slab-add
tree on the (otherwise slack) DVE reduces the kt tiles of each step's
P^T to a [128,512] partial, and a single [128,128] all-ones matmul per
step finishes the cross-partition sum with the result already
broadcast to every partition row, feeding reciprocal directly (no
GpSimd partition_broadcast).  The tree for step s+1 runs on DVE under
step s's PE work.  Stale [0:off] columns of diagonal P^T tiles are
memset at ring-allocation time so the slab adds read zeros there.

The four (wq, win2) projection passes (outputs first needed by the g=2
steps) are deferred out of the projection phase and emitted as dense
PE filler inside attention steps 1..4, which otherwise idle on the
ACT-paced exp pipeline (no out-proj tiles exist until the first group
completes); wq and the win-2 slice of x^T stay SBUF-resident through
stage 2 to support this.  Out-proj tiles are interleaved as filler in
later steps; PSUM->SBUF out copies alternate DVE/ACT; partial outputs
are written bf16 (host sums partials in fp64).

Matmul inputs are bf16 (fp32 accumulation in PSUM); measured end-to-end
relative error vs the fp32 reference is ~5.7e-3 (gate 2e-2).

NOTE on run-to-run variance: the chip's power management sometimes
pins the PE at ~1.95 GHz instead of 2.4 (P0 downclock), inflating any
single measurement by ~19%.  Back-to-back runs without cooldown make
this more likely; the ~313 us figure reproduces consistently with a
~20 s idle gap between runs.
"""

import math

import numpy as np
import ml_dtypes

import concourse.bass as bass
import concourse.mybir as mybir
import concourse.tile as tile
from concourse import bacc
from concourse.bass_utils import run_bass_kernel_spmd

BF16 = mybir.dt.bfloat16
F32 = mybir.dt.float32
ALU = mybir.AluOpType
ACT_EXP = mybir.ActivationFunctionType.Exp

P = 128              # partitions
D_IN = 2048          # model dim
N_SEQ = 2048         # sequence length
HD = 128             # head dim
HPC = 4              # heads per core
DC = HPC * HD        # 512: d_out slice per core
N_CORES = 8
SCALE = 1.0 / math.sqrt(HD)
NEG_BIG = -1e10

NT = N_SEQ // P      # 16 seq tiles of 128
NI = D_IN // P       # 16 contraction chunks of 128
NG = NT // 4         # 4 groups of 4 q-tiles (q-512 groups)
NJ = D_IN // 512     # 4 output column chunks


def _build_body(tc, xt_d, wq_d, wk_d, wv_d, wo_d, out_d):
    nc = tc.nc
    from contextlib import ExitStack
    ctx = ExitStack()
    with ctx:
        # ---------------- constants ----------------
        const = ctx.enter_context(tc.tile_pool(name="const", bufs=1))
        # all-ones [128,128] lhsT: the denominator matmul ones.T @ acc then
        # emits the column sums already broadcast to every partition row
        ones_sb = const.tile([P, P], BF16)
        nc.vector.memset(ones_sb, 1.0)
        warmsrc = const.tile([P, 512], BF16)
        nc.vector.memset(warmsrc, 0.0)
        # force the exp activation table load at t=0, off the critical path
        tscr = const.tile([P, 1], F32)
        nc.vector.memset(tscr, 0.0)
        nc.scalar.activation(out=tscr, in_=tscr, func=ACT_EXP, bias=0.0, scale=1.0)

        # ---------------- persistent activations ----------------
        persist = ctx.enter_context(tc.tile_pool(name="persist", bufs=1))
        qt_sb = persist.tile([P, HPC, N_SEQ], BF16)    # Q^T  [d, h, n]
        kt_sb = persist.tile([P, HPC, N_SEQ], BF16)    # K^T  [d, h, n]
        v_sb = persist.tile([P, NT, DC], BF16)         # V natural [n(128), nt, d]
        ctxT_sb = persist.tile([P, HPC, N_SEQ], BF16)  # ctx^T [d, h, n]
        # wq and the win-2 slice of x^T stay resident through stage 2: the
        # four (wq, win2) projection passes are deferred into the first
        # attention steps as dense PE filler (their outputs are only needed
        # by the g=2 steps)
        wq_sb = persist.tile([P, NI, DC], BF16)
        xt2_sb = persist.tile([P, NI, 512], BF16)
        # dedicated P^T tile for the score step woven into projection
        wv_ptg0 = persist.tile([P, NT, 512], BF16)
        # the denominator tree slab-adds read full [128,512] kt rows, so the
        # never-written [0:off] columns of diagonal tiles must be zeroed
        def memset_diag_holes(g, ptg):
            for kt in range(4 * g + 1, 4 * (g + 1)):
                off = (kt - 4 * g) * P
                nc.vector.memset(ptg[:, kt, 0:off], 0.0)

        memset_diag_holes(3, wv_ptg0)

        xt_r = xt_d.rearrange("(io p) nn -> p io nn", p=P)
        wq_r = wq_d.rearrange("(io p) c -> p io c", p=P)
        wk_r = wk_d.rearrange("(io p) c -> p io c", p=P)
        wv_r = wv_d.rearrange("(io p) c -> p io c", p=P)
        wo_r = wo_d.rearrange("(h p) j -> p h j", p=P)

        # step list: groups in descending size so the pipeline has big work
        # first; (g, h) pairs in emission order
        steps = [(3 - i // HPC, i % HPC) for i in range(NG * HPC)]
        LAG = 2

        # ---------------- score/exp emitter (shared proj + main) ----------
        def emit_scores_units(g, h, ptg, s_pool):
            """Yield after each PE unit: full-kt pairs then diag pairs.
            Scores for group g head h: S^T = K_kt.T @ Q^T, mask accumulated
            on the PE for diagonal blocks, batched exp into ptg (bf16)."""
            nkt = 4 * (g + 1)
            q0 = 512 * g
            nfull = 4 * g
            # full-width kt tiles, two per [128,1024] psum tensor with one
            # batched exp (amortizes ACT access latency)
            for kp in range(0, nfull, 2):
                sps = s_pool.tile([P, 1024], F32, tag="s", name="sps")
                for j, kt in enumerate((kp, kp + 1)):
                    nc.tensor.matmul(
                        sps[:, j * 512:(j + 1) * 512],
                        lhsT=kt_sb[:, h, kt * P:(kt + 1) * P],
                        rhs=qt_sb[:, h, q0:q0 + 512],
                        start=True, stop=True,
                    )
                nc.scalar.activation(
                    out=ptg[:, kp:kp + 2, :], in_=sps,
                    func=ACT_EXP, bias=0.0, scale=SCALE,
                )
                yield
            # diagonal kt tiles (off = 128*(kt-4g)), two per psum tensor;
            # exp the full block (no overflow: |s*scale| <= ~9), then zero the
            # sub-diagonal triangle of P^T on the idle GpSimd engine
            for kp in range(nfull, nkt, 2):
                sps = s_pool.tile([P, 1024], F32, tag="s", name="sps")
                for j, kt in enumerate((kp, kp + 1)):
                    off = (kt - nfull) * P
                    nc.tensor.matmul(
                        sps[:, j * 512 + off:(j + 1) * 512],
                        lhsT=kt_sb[:, h, kt * P:(kt + 1) * P],
                        rhs=qt_sb[:, h, q0 + off:q0 + 512],
                        start=True, stop=True,
                    )
                for j, kt in enumerate((kp, kp + 1)):
                    off = (kt - nfull) * P
                    nc.scalar.activation(
                        out=ptg[:, kt, off:512],
                        in_=sps[:, j * 512 + off:(j + 1) * 512],
                        func=ACT_EXP, bias=0.0, scale=SCALE,
                    )
                for j, kt in enumerate((kp, kp + 1)):
                    off = (kt - nfull) * P
                    nc.gpsimd.affine_select(
                        out=ptg[:, kt, off:off + P], in_=ptg[:, kt, off:off + P],
                        compare_op=ALU.is_ge, fill=0.0,
                        base=0, pattern=[[1, P]], channel_multiplier=-1,
                    )
                yield

        # ---------------- stage 1: projections (V first, then Q/K) -------
        # xt_sb holds windows 0,1,3 (slots 0,1,2); window 2 lives in the
        # persistent xt2_sb so the deferred (wq, win2) passes can run during
        # the attention phase
        WSLOT = {0: 0, 1: 1, 3: 2}

        with tc.tile_pool(name="xw", bufs=1) as xw_pool, \
             tc.tile_pool(name="wqk", bufs=1) as wqk_pool, \
             tc.tile_pool(name="pjps", bufs=2, space="PSUM") as pj_psum:
            xt_sb = xw_pool.tile([P, NI, 3 * 512], BF16)
            wk_sb = wqk_pool.tile([P, NI, DC], BF16)

            def xt_win(i, win):
                if win == 2:
                    return xt2_sb[:, i, :]
                s = WSLOT[win] * 512
                return xt_sb[:, i, s:s + 512]

            def xt_nt(i, nt):
                win, o = divmod(nt * P, 512)
                src = xt2_sb if win == 2 else xt_sb
                s = (o if win == 2 else WSLOT[win] * 512 + o)
                return src[:, i, s:s + P]

            # input DMAs in consumption order, ~0.26MB chunks (2 i-rows x
            # 512 cols) so the 16 DMA engines run in parallel
            def dma_w(dst, src):
                for i0 in range(0, NI, 2):
                    nc.sync.dma_start(dst[:, i0:i0 + 2, :], src[:, i0:i0 + 2, :])

            def dma_xt_win(w, fine=False):
                c0, c1 = w * 512, (w + 1) * 512
                step = 1 if fine else 2
                for i0 in range(0, NI, step):
                    if w == 2:
                        nc.sync.dma_start(xt2_sb[:, i0:i0 + step, :],
                                          xt_r[:, i0:i0 + step, c0:c1])
                    else:
                        s = WSLOT[w] * 512
                        nc.sync.dma_start(xt_sb[:, i0:i0 + step, s:s + 512],
                                          xt_r[:, i0:i0 + step, c0:c1])

            def emit_qk_pass(h, w_sb, dst, win):
                ps = pj_psum.tile([P, 512], F32, tag="v", bufs=4, name="qkps")
                for i in range(NI):
                    nc.tensor.matmul(
                        ps,
                        lhsT=w_sb[:, i, h * P:(h + 1) * P],
                        rhs=xt_win(i, win),
                        start=(i == 0), stop=(i == NI - 1),
                    )
                nc.vector.tensor_copy(
                    out=dst[:, h, win * 512:(win + 1) * 512], in_=ps)

            # wv lives in an inner scope so its SBUF frees before the weave
            with tc.tile_pool(name="wvp", bufs=1) as wv_pool:
                wv_sb = wv_pool.tile([P, NI, DC], BF16)
                # first round: wv and xt-win0 i-chunks interleaved in ~0.13MB
                # units so i=0..7 of BOTH land on the first pass of all 16
                # DMA queues (~6us) and the V projection can start early
                for i in range(0, NI, 2):
                    nc.sync.dma_start(wv_sb[:, i:i + 1, :], wv_r[:, i:i + 1, :])
                    nc.sync.dma_start(xt_sb[:, i:i + 1, 0:512],
                                      xt_r[:, i:i + 1, 0:512])
                    nc.sync.dma_start(wv_sb[:, i + 1:i + 2, :],
                                      wv_r[:, i + 1:i + 2, :])
                    nc.sync.dma_start(xt_sb[:, i + 1:i + 2, 0:512],
                                      xt_r[:, i + 1:i + 2, 0:512])
                # (slot mapping makes window 0 land at cols 0:512 above)
                dma_xt_win(1, fine=True)
                dma_w(wq_sb, wq_r)
                dma_w(wk_sb, wk_r)
                dma_xt_win(2)
                dma_xt_win(3)

                # dummy matmuls: keep the PE busy (and ramp the HAM clock
                # gate to full speed) while the first DMAs land
                warm_ps = pj_psum.tile([P, 512], F32, tag="v", bufs=4,
                                       name="warm_ps")
                for _ in range(14):
                    nc.tensor.matmul(warm_ps, lhsT=warmsrc[:, :P], rhs=warmsrc,
                                     start=True, stop=True)

                def emit_v_batch(nts):
                    """Four nt chains interleaved over i so the PE can chew
                    every i-chunk as soon as its DMA lands (the first rounds
                    arrive in i=0..7-of-everything order)."""
                    pss = {nt: pj_psum.tile([P, DC], F32, tag="v", bufs=4,
                                            name=f"vps{nt}") for nt in nts}
                    for i in range(NI):
                        for nt in nts:
                            nc.tensor.matmul(
                                pss[nt],
                                lhsT=xt_nt(i, nt),
                                rhs=wv_sb[:, i, :],
                                start=(i == 0), stop=(i == NI - 1),
                                skip_group_check=True,
                            )
                    for nt in nts:
                        nc.vector.tensor_copy(out=v_sb[:, nt, :], in_=pss[nt])

                # V-A, QK-A (windows 0,1), V-B
                emit_v_batch([0, 1, 2, 3])
                emit_v_batch([4, 5, 6, 7])
                for h in range(HPC):
                    for w_sb, dst in ((wq_sb, qt_sb), (wk_sb, kt_sb)):
                        for win in (0, 1):
                            emit_qk_pass(h, w_sb, dst, win)
                emit_v_batch([8, 9, 10, 11])
                emit_v_batch([12, 13, 14, 15])

            # QK-B (windows 2,3) with the first two g=3 score steps woven in
            weave = []  # generators of woven score units

            def pump_weave(n):
                for _ in range(n):
                    for gen in list(weave):
                        try:
                            next(gen)
                            break
                        except StopIteration:
                            weave.remove(gen)

            for h in range(HPC):
                for w_sb, dst, win in ((wq_sb, qt_sb, 3), (wk_sb, kt_sb, 2),
                                       (wk_sb, kt_sb, 3)):
                    # (wq, win2) passes are deferred into the attention steps
                    emit_qk_pass(h, w_sb, dst, win)
                    pump_weave(3)
                if h == 0:
                    weave.append(emit_scores_units(3, 0, wv_ptg0, pj_psum))
            pump_weave(100)

        # ---------------- stage 2: attention + out-projection ------------
        with tc.tile_pool(name="att", bufs=3) as att_pool, \
             tc.tile_pool(name="small", bufs=2) as small_pool, \
             tc.tile_pool(name="tsum", bufs=2) as tsum_pool, \
             tc.tile_pool(name="osb", bufs=4) as out_pool, \
             tc.tile_pool(name="sps2", bufs=2, space="PSUM") as s_pool, \
             tc.tile_pool(name="colps", bufs=1, space="PSUM") as col_pool, \
             tc.tile_pool(name="ops", bufs=3, space="PSUM") as o_pool:
            wo_sb = att_pool.tile([P, HPC, D_IN], BF16, tag="wo", bufs=1)
            for hh in range(HPC):
                for j0 in (0, 1024):
                    nc.sync.dma_start(wo_sb[:, hh, j0:j0 + 1024],
                                      wo_r[:, hh, j0:j0 + 1024])

            ptgs = {(3, 0): wv_ptg0}

            def emit_tree(g, h, ptg):
                """Pairwise bf16 slab-add tree over the nkt exp'd P^T tiles
                on the (non-critical) DVE; replaces per-kt ones-matmuls on
                the PE.  Returns the [128,512] per-partition partial sums."""
                nkt = 4 * (g + 1)
                m = nkt // 2
                sc = tsum_pool.tile([P, 8, 512], BF16, tag="ts", name="tsc")
                nc.vector.tensor_tensor(out=sc[:, 0:m, :], in0=ptg[:, 0:m, :],
                                        in1=ptg[:, m:nkt, :], op=ALU.add)
                while m > 1:
                    if m % 2 == 0:
                        m //= 2
                        nc.vector.tensor_tensor(
                            out=sc[:, 0:m, :], in0=sc[:, 0:m, :],
                            in1=sc[:, m:2 * m, :], op=ALU.add)
                    else:  # m == 3
                        nc.vector.tensor_tensor(
                            out=sc[:, 0:1, :], in0=sc[:, 0:1, :],
                            in1=sc[:, 1:2, :], op=ALU.add)
                        nc.vector.tensor_tensor(
                            out=sc[:, 0:1, :], in0=sc[:, 0:1, :],
                            in1=sc[:, 2:3, :], op=ALU.add)
                        m = 1
                return sc

            def emit_den(acc):
                """One ones.T @ acc matmul finishes the column sums over the
                partition axis, already broadcast to all 128 rows; reciprocal
                feeds the fused ctx normalize directly."""
                colp = col_pool.tile([P, 512], F32, tag="col", name="colp")
                nc.tensor.matmul(colp, lhsT=ones_sb, rhs=acc[:, 0, :],
                                 start=True, stop=True)
                recip_bc = small_pool.tile([P, 512], F32, tag="rsb",
                                           name="recip_bc")
                nc.vector.reciprocal_approx_fast(out=recip_bc, in_=colp)
                return recip_bc

            def emit_ctx(g, h, ptg, recip_bc):
                """ctx^T accumulated over k tiles, 1/colsum fused into the
                PSUM->SBUF copy."""
                nkt = 4 * (g + 1)
                cps = o_pool.tile([P, 512], F32, tag="o", name="cps")
                for kt in range(nkt):
                    off = max(kt - 4 * g, 0) * P
                    nc.tensor.matmul(
                        cps[:, off:512],
                        lhsT=v_sb[:, kt, h * P:(h + 1) * P],
                        rhs=ptg[:, kt, off:512],
                        start=(kt == 0), stop=(kt == nkt - 1),
                        skip_group_check=True,
                    )
                nc.vector.tensor_tensor(
                    out=ctxT_sb[:, h, g * 512:(g + 1) * 512],
                    in0=cps, in1=recip_bc, op=ALU.mult,
                )

            def emit_qkdef_units(h):
                """Deferred (wq, win2) projection pass for head h: dense PE
                filler for the early attention steps (which have no out-proj
                tiles yet).  Yields every 4 matmuls."""
                ps = col_pool.tile([P, 512], F32, tag="col", name="qkdps")
                for i in range(NI):
                    nc.tensor.matmul(
                        ps,
                        lhsT=wq_sb[:, i, h * P:(h + 1) * P],
                        rhs=xt2_sb[:, i, :],
                        start=(i == 0), stop=(i == NI - 1),
                        skip_group_check=True,
                    )
                    if i % 4 == 3 and i < NI - 1:
                        yield
                nc.vector.tensor_copy(out=qt_sb[:, h, 1024:1536], in_=ps)
                yield

            op_queue = []  # ready out-projection (nt, jc) tiles
            op_count = [0]

            def emit_op_tile(allow_act=False):
                nt, jc = op_queue.pop(0)
                ops = o_pool.tile([P, 512], F32, tag="o", name="ops")
                for hh in range(HPC):
                    nc.tensor.matmul(
                        ops,
                        lhsT=ctxT_sb[:, hh, nt * P:(nt + 1) * P],
                        rhs=wo_sb[:, hh, jc * 512:(jc + 1) * 512],
                        start=(hh == 0), stop=(hh == HPC - 1),
                    )
                osb = out_pool.tile([P, 512], BF16, tag="osb", name="osb")
                op_count[0] += 1
                if allow_act and op_count[0] % 2 == 0:
                    nc.scalar.copy(out=osb, in_=ops)
                else:
                    nc.vector.tensor_copy(out=osb, in_=ops)
                nc.sync.dma_start(
                    out_d[nt * P:(nt + 1) * P, jc * 512:(jc + 1) * 512], osb)

            next_sc = 1  # (3,0) was woven into projection
            trees = {0: emit_tree(3, 0, wv_ptg0)}
            defgens = []  # deferred (wq, win2) pass generators
            for idx, (g, h) in enumerate(steps):
                if 1 <= idx <= HPC:
                    defgens.append(emit_qkdef_units(idx - 1))
                # emit scores up to LAG steps ahead, interleaved below
                gens = []
                while next_sc <= min(idx + LAG, len(steps) - 1):
                    g2, h2 = steps[next_sc]
                    ptg2 = att_pool.tile([P, NT, 512], BF16, tag="ptg",
                                         name=f"ptg_{g2}_{h2}")
                    memset_diag_holes(g2, ptg2)
                    ptgs[(g2, h2)] = ptg2
                    gens.append(emit_scores_units(g2, h2, ptg2, s_pool))
                    next_sc += 1

                def pump(n=1):
                    while n > 0 and gens:
                        try:
                            next(gens[0])
                            n -= 1
                        except StopIteration:
                            gens.pop(0)

                budget = 4 if idx < 8 else 7
                ptg = ptgs.pop((g, h))
                # ready score matmuls first so the PE never waits on the
                # denominator tree finishing on DVE
                pump(2)
                # denominator tree for the NEXT step (its P^T finished during
                # the previous step's pumps); runs on DVE under this step's
                # PE work.  At idx 0 the (3,1) P^T isn't done yet - deferred.
                if idx > 0 and idx + 1 < len(steps):
                    g2, h2 = steps[idx + 1]
                    trees[idx + 1] = emit_tree(g2, h2, ptgs[(g2, h2)])
                recip_bc = emit_den(trees.pop(idx))
                emit_ctx(g, h, ptg, recip_bc)
                # interleave remaining score units with dense filler:
                # deferred (wq, win2) projection chunks, then out-proj tiles
                for _ in range(10):
                    pump(1)
                    if defgens:
                        try:
                            next(defgens[0])
                        except StopIteration:
                            defgens.pop(0)
                    elif budget > 0 and op_queue:
                        emit_op_tile(allow_act=(idx >= 6))
                        budget -= 1
                pump(100)
                while defgens:  # finish any deferred pass inside this step
                    try:
                        next(defgens[0])
                    except StopIteration:
                        defgens.pop(0)
                while budget > 0 and op_queue:
                    emit_op_tile(allow_act=(idx >= 6))
                    budget -= 1
                if idx == 0:
                    trees[1] = emit_tree(*steps[1], ptgs[steps[1]])
                if h == HPC - 1:
                    op_queue.extend(
                        (nt, jc) for nt in range(4 * g, 4 * g + 4)
                        for jc in range(NJ))
            while op_queue:
                emit_op_tile(allow_act=True)


def build_module():
    """Build and compile the per-core Bass module (SPMD: same program, 8 cores)."""
    nc = bacc.Bacc("TRN2", target_bir_lowering=False, debug=False,
                   num_devices=N_CORES)
    xt_d = nc.dram_tensor("xt", [D_IN, N_SEQ], BF16, kind="ExternalInput").ap()
    wq_d = nc.dram_tensor("wq", [D_IN, DC], BF16, kind="ExternalInput").ap()
    wk_d = nc.dram_tensor("wk", [D_IN, DC], BF16, kind="ExternalInput").ap()
    wv_d = nc.dram_tensor("wv", [D_IN, DC], BF16, kind="ExternalInput").ap()
    wo_d = nc.dram_tensor("wo", [DC, D_IN], BF16, kind="ExternalInput").ap()
    out_d = nc.dram_tensor("out", [N_SEQ, D_IN], BF16, kind="ExternalOutput").ap()
    with tile.TileContext(nc) as tc:
        _build_body(tc, xt_d, wq_d, wk_d, wv_d, wo_d, out_d)
    nc.compile()
    return nc


def make_in_maps(x, W_qkv, W_out):
    """Host-side sharding: per-core input dict, bf16 cast + pre-transposed x."""
    bf = ml_dtypes.bfloat16
    in_maps = []
    for c in range(N_CORES):
        b, g = divmod(c, 4)
        cs = slice(DC * g, DC * (g + 1))
        in_maps.append({
            "xt": np.ascontiguousarray(x[b].T).astype(bf),
            "wq": np.ascontiguousarray(W_qkv[:, 0 * D_IN:1 * D_IN][:, cs]).astype(bf),
            "wk": np.ascontiguousarray(W_qkv[:, 1 * D_IN:2 * D_IN][:, cs]).astype(bf),
            "wv": np.ascontiguousarray(W_qkv[:, 2 * D_IN:3 * D_IN][:, cs]).astype(bf),
            "wo": np.ascontiguousarray(W_out[cs, :]).astype(bf),
        })
    return in_maps


_NC_CACHE = {}


def get_module():
    if "nc" not in _NC_CACHE:
        _NC_CACHE["nc"] = build_module()
    return _NC_CACHE["nc"]


def run(x, W_qkv, W_out, b_out, trace=False, **trace_kwargs):
    nc = get_module()
    in_maps = make_in_maps(x, W_qkv, W_out)
    res = run_bass_kernel_spmd(nc, in_maps, core_ids=list(range(N_CORES)),
                               trace=trace, **trace_kwargs)
    parts = np.stack([np.asarray(res.results[c]["out"], dtype=np.float32)
                      for c in range(N_CORES)])
    parts = parts.reshape(2, 4, N_SEQ, D_IN)
    out = parts.sum(axis=1, dtype=np.float64).astype(np.float32)
    out += b_out.astype(np.float32)
    return out, res


def kernel(x, W_qkv, W_out, b_out):
    out, _ = run(np.asarray(x), np.asarray(W_qkv), np.asarray(W_out),
                 np.asarray(b_out))
    return out



# revision 50
# speedup vs baseline: 1.2302x; 1.0022x over previous
"""Multi-head causal self-attention (B=2, N=2048, D=2048, H=16) on 8 NeuronCores.

Sharding: core c handles batch b = c//4 and heads 4*(c%4) .. 4*(c%4)+3
(data parallel over batch, tensor parallel over heads).  Each core:
  - projects V first (natural [seq, head_dim] layout), then Q^T / K^T in
    [head_dim, seq] layout, contracting over d_in chunks that sit on
    partitions (x is pre-transposed on the host),
  - runs causal attention per head entirely in transposed space
    (S^T = K_tile Q^T, exp on ScalarE writes P^T straight into SBUF,
    denominators via a ones-row matmul, 1/sum fused into the ctx copy),
  - computes the partial output projection ctx_slice @ W_out[rows_slice]
    into a [2048, 2048] fp32 partial.
The host sums the 4 partials per batch and adds the output bias.

Schedule (v12, ~310 us vs the 390 us v1 / 347 us v7 baselines at the
same ~2.32 GHz effective PE clock): input DMAs are issued in
consumption order (first rounds as ~0.13 MB i-chunks of wv and xt-win0
interleaved across the 16 DMA queues so the 4-way-interleaved V
projection can start as soon as the first round lands); the g=3 head-0
score block + exp is woven into the tail of the QK projection; scores
for step s+2 are emitted as interleaved units inside step s; exp is
batched over [128,1024] PSUM pairs; the causal mask is applied by
zeroing the sub-diagonal triangle of P^T on the GpSimd engine after
exp (exp cannot overflow: |s*scale| <= ~9).

Softmax denominators are NOT computed with per-kt ones-row matmuls on
the PE (that costs a second full pass of P^T through the array, ~29 us
of the 128-wide PE stream at M=1): instead a pairwise bf16 slab-add
tree on the (otherwise slack) DVE reduces the kt tiles of each step's
P^T to a [128,512] partial, and a single [128,128] all-ones matmul per
step finishes the cross-partition sum with the result already
broadcast to every partition row, feeding reciprocal directly (no
GpSimd partition_broadcast).  The tree for step s+1 runs on DVE under
step s's PE work.  Stale [0:off] columns of diagonal P^T tiles are
memset at ring-allocation time so the slab adds read zeros there.

The four (wq, win2) projection passes (outputs first needed by the g=2
steps) are deferred out of the projection phase and emitted as dense
PE filler inside attention steps 1..4, which otherwise idle on the
ACT-paced exp pipeline (no out-proj tiles exist until the first group
completes); wq and the win-2 slice of x^T stay SBUF-resident through
stage 2 to support this.  Out-proj tiles are interleaved as filler in
later steps; PSUM->SBUF out copies alternate DVE/ACT; partial outputs
are written bf16 (host sums partials in fp64).

Matmul inputs are bf16 (fp32 accumulation in PSUM); measured end-to-end
relative error vs the fp32 reference is ~5.7e-3 (gate 2e-2).

NOTE on run-to-run variance: the chip's power management sometimes
pins the PE at ~1.95 GHz instead of 2.4 (P0 downclock), inflating any
single measurement by ~19%.  Back-to-back runs without cooldown make
this more likely; the ~310 us figure reproduces consistently with a
~20 s idle gap between runs.
"""

import math

import numpy as np
import ml_dtypes

import concourse.bass as bass
import concourse.mybir as mybir
import concourse.tile as tile
from concourse import bacc
from concourse.bass_utils import run_bass_kernel_spmd

BF16 = mybir.dt.bfloat16
F32 = mybir.dt.float32
ALU = mybir.AluOpType
ACT_EXP = mybir.ActivationFunctionType.Exp

P = 128              # partitions
D_IN = 2048          # model dim
N_SEQ = 2048         # sequence length
HD = 128             # head dim
HPC = 4              # heads per core
DC = HPC * HD        # 512: d_out slice per core
N_CORES = 8
SCALE = 1.0 / math.sqrt(HD)
NEG_BIG = -1e10

NT = N_SEQ // P      # 16 seq tiles of 128
NI = D_IN // P       # 16 contraction chunks of 128
NG = NT // 4         # 4 groups of 4 q-tiles (q-512 groups)
NJ = D_IN // 512     # 4 output column chunks


def _build_body(tc, xt_d, wq_d, wk_d, wv_d, wo_d, out_d):
    nc = tc.nc
    from contextlib import ExitStack
    ctx = ExitStack()
    with ctx:
        # ---------------- constants ----------------
        const = ctx.enter_context(tc.tile_pool(name="const", bufs=1))
        # all-ones [128,128] lhsT: the denominator matmul ones.T @ acc then
        # emits the column sums already broadcast to every partition row
        ones_sb = const.tile([P, P], BF16)
        nc.vector.memset(ones_sb, 1.0)
        warmsrc = const.tile([P, 512], BF16)
        nc.vector.memset(warmsrc, 0.0)
        # force the exp activation table load at t=0, off the critical path
        tscr = const.tile([P, 1], F32)
        nc.vector.memset(tscr, 0.0)
        nc.scalar.activation(out=tscr, in_=tscr, func=ACT_EXP, bias=0.0, scale=1.0)

        # score-psum ring shared by the projection-phase weave and the
        # attention steps: no pool boundary at the stage transition, so the
        # first stage-2 score matmul flows straight after the weave
        s_pool = ctx.enter_context(
            tc.tile_pool(name="spool", bufs=2, space="PSUM"))

        # ---------------- persistent activations ----------------
        persist = ctx.enter_context(tc.tile_pool(name="persist", bufs=1))
        qt_sb = persist.tile([P, HPC, N_SEQ], BF16)    # Q^T  [d, h, n]
        kt_sb = persist.tile([P, HPC, N_SEQ], BF16)    # K^T  [d, h, n]
        v_sb = persist.tile([P, NT, DC], BF16)         # V natural [n(128), nt, d]
        ctxT_sb = persist.tile([P, HPC, N_SEQ], BF16)  # ctx^T [d, h, n]
        # wq and the win-2 slice of x^T stay resident through stage 2: the
        # four (wq, win2) projection passes are deferred into the first
        # attention steps as dense PE filler (their outputs are only needed
        # by the g=2 steps)
        wq_sb = persist.tile([P, NI, DC], BF16)
        xt2_sb = persist.tile([P, NI, 512], BF16)
        # dedicated P^T tile for the score step woven into projection
        wv_ptg0 = persist.tile([P, NT, 512], BF16)
        # the denominator tree slab-adds read full [128,512] kt rows, so the
        # never-written [0:off] columns of diagonal tiles must be zeroed
        def memset_diag_holes(g, ptg):
            for kt in range(4 * g + 1, 4 * (g + 1)):
                off = (kt - 4 * g) * P
                nc.vector.memset(ptg[:, kt, 0:off], 0.0)

        memset_diag_holes(3, wv_ptg0)

        xt_r = xt_d.rearrange("(io p) nn -> p io nn", p=P)
        wq_r = wq_d.rearrange("(io p) c -> p io c", p=P)
        wk_r = wk_d.rearrange("(io p) c -> p io c", p=P)
        wv_r = wv_d.rearrange("(io p) c -> p io c", p=P)
        wo_r = wo_d.rearrange("(h p) j -> p h j", p=P)

        # step list: groups in descending size so the pipeline has big work
        # first; (g, h) pairs in emission order
        steps = [(3 - i // HPC, i % HPC) for i in range(NG * HPC)]
        LAG = 2

        # ---------------- score/exp emitter (shared proj + main) ----------
        def emit_scores_units(g, h, ptg, s_pool):
            """Yield after each PE unit: full-kt pairs then diag pairs.
            Scores for group g head h: S^T = K_kt.T @ Q^T, mask accumulated
            on the PE for diagonal blocks, batched exp into ptg (bf16)."""
            nkt = 4 * (g + 1)
            q0 = 512 * g
            nfull = 4 * g
            # full-width kt tiles, two per [128,1024] psum tensor with one
            # batched exp (amortizes ACT access latency)
            for kp in range(0, nfull, 2):
                sps = s_pool.tile([P, 1024], F32, tag="s", name="sps")
                for j, kt in enumerate((kp, kp + 1)):
                    nc.tensor.matmul(
                        sps[:, j * 512:(j + 1) * 512],
                        lhsT=kt_sb[:, h, kt * P:(kt + 1) * P],
                        rhs=qt_sb[:, h, q0:q0 + 512],
                        start=True, stop=True,
                    )
                nc.scalar.activation(
                    out=ptg[:, kp:kp + 2, :], in_=sps,
                    func=ACT_EXP, bias=0.0, scale=SCALE,
                )
                yield
            # diagonal kt tiles (off = 128*(kt-4g)), two per psum tensor;
            # exp the full block (no overflow: |s*scale| <= ~9), then zero the
            # sub-diagonal triangle of P^T on the idle GpSimd engine
            for kp in range(nfull, nkt, 2):
                sps = s_pool.tile([P, 1024], F32, tag="s", name="sps")
                for j, kt in enumerate((kp, kp + 1)):
                    off = (kt - nfull) * P
                    nc.tensor.matmul(
                        sps[:, j * 512 + off:(j + 1) * 512],
                        lhsT=kt_sb[:, h, kt * P:(kt + 1) * P],
                        rhs=qt_sb[:, h, q0 + off:q0 + 512],
                        start=True, stop=True,
                    )
                for j, kt in enumerate((kp, kp + 1)):
                    off = (kt - nfull) * P
                    nc.scalar.activation(
                        out=ptg[:, kt, off:512],
                        in_=sps[:, j * 512 + off:(j + 1) * 512],
                        func=ACT_EXP, bias=0.0, scale=SCALE,
                    )
                for j, kt in enumerate((kp, kp + 1)):
                    off = (kt - nfull) * P
                    nc.gpsimd.affine_select(
                        out=ptg[:, kt, off:off + P], in_=ptg[:, kt, off:off + P],
                        compare_op=ALU.is_ge, fill=0.0,
                        base=0, pattern=[[1, P]], channel_multiplier=-1,
                    )
                yield

        # ---------------- stage 1: projections (V first, then Q/K) -------
        # xt_sb holds windows 0,1,3 (slots 0,1,2); window 2 lives in the
        # persistent xt2_sb so the deferred (wq, win2) passes can run during
        # the attention phase
        WSLOT = {0: 0, 1: 1, 3: 2}

        with tc.tile_pool(name="xw", bufs=1) as xw_pool, \
             tc.tile_pool(name="wqk", bufs=1) as wqk_pool, \
             tc.tile_pool(name="pjps", bufs=2, space="PSUM") as pj_psum:
            xt_sb = xw_pool.tile([P, NI, 3 * 512], BF16)
            wk_sb = wqk_pool.tile([P, NI, DC], BF16)

            def xt_win(i, win):
                if win == 2:
                    return xt2_sb[:, i, :]
                s = WSLOT[win] * 512
                return xt_sb[:, i, s:s + 512]

            def xt_nt(i, nt):
                win, o = divmod(nt * P, 512)
                src = xt2_sb if win == 2 else xt_sb
                s = (o if win == 2 else WSLOT[win] * 512 + o)
                return src[:, i, s:s + P]

            # input DMAs in consumption order, ~0.26MB chunks (2 i-rows x
            # 512 cols) so the 16 DMA engines run in parallel
            def dma_w(dst, src):
                for i0 in range(0, NI, 2):
                    nc.sync.dma_start(dst[:, i0:i0 + 2, :], src[:, i0:i0 + 2, :])

            def dma_xt_win(w, fine=False):
                c0, c1 = w * 512, (w + 1) * 512
                step = 1 if fine else 2
                for i0 in range(0, NI, step):
                    if w == 2:
                        nc.sync.dma_start(xt2_sb[:, i0:i0 + step, :],
                                          xt_r[:, i0:i0 + step, c0:c1])
                    else:
                        s = WSLOT[w] * 512
                        nc.sync.dma_start(xt_sb[:, i0:i0 + step, s:s + 512],
                                          xt_r[:, i0:i0 + step, c0:c1])

            def emit_qk_pass(h, w_sb, dst, win):
                ps = pj_psum.tile([P, 512], F32, tag="v", bufs=4, name="qkps")
                for i in range(NI):
                    nc.tensor.matmul(
                        ps,
                        lhsT=w_sb[:, i, h * P:(h + 1) * P],
                        rhs=xt_win(i, win),
                        start=(i == 0), stop=(i == NI - 1),
                    )
                nc.vector.tensor_copy(
                    out=dst[:, h, win * 512:(win + 1) * 512], in_=ps)

            # wv lives in an inner scope so its SBUF frees before the weave
            with tc.tile_pool(name="wvp", bufs=1) as wv_pool:
                wv_sb = wv_pool.tile([P, NI, DC], BF16)
                # first round: wv and xt-win0 i-chunks interleaved in ~0.13MB
                # units so i=0..7 of BOTH land on the first pass of all 16
                # DMA queues (~6us) and the V projection can start early
                for i in range(0, NI, 2):
                    nc.sync.dma_start(wv_sb[:, i:i + 1, :], wv_r[:, i:i + 1, :])
                    nc.sync.dma_start(xt_sb[:, i:i + 1, 0:512],
                                      xt_r[:, i:i + 1, 0:512])
                    nc.sync.dma_start(wv_sb[:, i + 1:i + 2, :],
                                      wv_r[:, i + 1:i + 2, :])
                    nc.sync.dma_start(xt_sb[:, i + 1:i + 2, 0:512],
                                      xt_r[:, i + 1:i + 2, 0:512])
                # (slot mapping makes window 0 land at cols 0:512 above)
                dma_xt_win(1, fine=True)
                dma_w(wq_sb, wq_r)
                dma_w(wk_sb, wk_r)
                dma_xt_win(2)
                dma_xt_win(3)

                # dummy matmuls: keep the PE busy (and ramp the HAM clock
                # gate to full speed) while the first DMAs land
                warm_ps = pj_psum.tile([P, 512], F32, tag="v", bufs=4,
                                       name="warm_ps")
                for _ in range(14):
                    nc.tensor.matmul(warm_ps, lhsT=warmsrc[:, :P], rhs=warmsrc,
                                     start=True, stop=True)

                def emit_v_batch(nts):
                    """Four nt chains interleaved over i so the PE can chew
                    every i-chunk as soon as its DMA lands (the first rounds
                    arrive in i=0..7-of-everything order)."""
                    pss = {nt: pj_psum.tile([P, DC], F32, tag="v", bufs=4,
                                            name=f"vps{nt}") for nt in nts}
                    for i in range(NI):
                        for nt in nts:
                            nc.tensor.matmul(
                                pss[nt],
                                lhsT=xt_nt(i, nt),
                                rhs=wv_sb[:, i, :],
                                start=(i == 0), stop=(i == NI - 1),
                                skip_group_check=True,
                            )
                    for nt in nts:
                        nc.vector.tensor_copy(out=v_sb[:, nt, :], in_=pss[nt])

                # V-A, QK-A (windows 0,1), V-B
                emit_v_batch([0, 1, 2, 3])
                emit_v_batch([4, 5, 6, 7])
                for h in range(HPC):
                    for w_sb, dst in ((wq_sb, qt_sb), (wk_sb, kt_sb)):
                        for win in (0, 1):
                            emit_qk_pass(h, w_sb, dst, win)
                emit_v_batch([8, 9, 10, 11])
                emit_v_batch([12, 13, 14, 15])

            # QK-B (windows 2,3) with the first two g=3 score steps woven in
            weave = []  # generators of woven score units

            def pump_weave(n):
                for _ in range(n):
                    for gen in list(weave):
                        try:
                            next(gen)
                            break
                        except StopIteration:
                            weave.remove(gen)

            for h in range(HPC):
                for w_sb, dst, win in ((wq_sb, qt_sb, 3), (wk_sb, kt_sb, 2),
                                       (wk_sb, kt_sb, 3)):
                    # (wq, win2) passes are deferred into the attention steps
                    emit_qk_pass(h, w_sb, dst, win)
                    pump_weave(3)
                if h == 0:
                    weave.append(emit_scores_units(3, 0, wv_ptg0, s_pool))
            pump_weave(100)

        # ---------------- stage 2: attention + out-projection ------------
        # PSUM pool order matters: col/o first so they land on the banks the
        # final QK-B pass (v tag) is still draining, and the score ring lands
        # on the long-finished weave banks - the first stage-2 score matmul
        # then never waits on the last QK-B cast
        with tc.tile_pool(name="att", bufs=3) as att_pool, \
             tc.tile_pool(name="small", bufs=2) as small_pool, \
             tc.tile_pool(name="tsum", bufs=2) as tsum_pool, \
             tc.tile_pool(name="osb", bufs=4) as out_pool, \
             tc.tile_pool(name="colps", bufs=1, space="PSUM") as col_pool, \
             tc.tile_pool(name="ops", bufs=3, space="PSUM") as o_pool:
            wo_sb = att_pool.tile([P, HPC, D_IN], BF16, tag="wo", bufs=1)
            for hh in range(HPC):
                for j0 in (0, 1024):
                    nc.sync.dma_start(wo_sb[:, hh, j0:j0 + 1024],
                                      wo_r[:, hh, j0:j0 + 1024])

            ptgs = {(3, 0): wv_ptg0}

            def emit_tree(g, h, ptg):
                """Pairwise bf16 slab-add tree over the nkt exp'd P^T tiles
                on the (non-critical) DVE; replaces per-kt ones-matmuls on
                the PE.  Returns the [128,512] per-partition partial sums."""
                nkt = 4 * (g + 1)
                m = nkt // 2
                sc = tsum_pool.tile([P, 8, 512], BF16, tag="ts", name="tsc")
                nc.vector.tensor_tensor(out=sc[:, 0:m, :], in0=ptg[:, 0:m, :],
                                        in1=ptg[:, m:nkt, :], op=ALU.add)
                while m > 1:
                    if m % 2 == 0:
                        m //= 2
                        nc.vector.tensor_tensor(
                            out=sc[:, 0:m, :], in0=sc[:, 0:m, :],
                            in1=sc[:, m:2 * m, :], op=ALU.add)
                    else:  # m == 3
                        nc.vector.tensor_tensor(
                            out=sc[:, 0:1, :], in0=sc[:, 0:1, :],
                            in1=sc[:, 1:2, :], op=ALU.add)
                        nc.vector.tensor_tensor(
                            out=sc[:, 0:1, :], in0=sc[:, 0:1, :],
                            in1=sc[:, 2:3, :], op=ALU.add)
                        m = 1
                return sc

            def emit_den(acc):
                """One ones.T @ acc matmul finishes the column sums over the
                partition axis, already broadcast to all 128 rows; reciprocal
                feeds the fused ctx normalize directly."""
                colp = col_pool.tile([P, 512], F32, tag="col", name="colp")
                nc.tensor.matmul(colp, lhsT=ones_sb, rhs=acc[:, 0, :],
                                 start=True, stop=True)
                recip_bc = small_pool.tile([P, 512], F32, tag="rsb",
                                           name="recip_bc")
                nc.vector.reciprocal_approx_fast(out=recip_bc, in_=colp)
                return recip_bc

            def emit_ctx(g, h, ptg, recip_bc):
                """ctx^T accumulated over k tiles, 1/colsum fused into the
                PSUM->SBUF copy."""
                nkt = 4 * (g + 1)
                cps = o_pool.tile([P, 512], F32, tag="o", name="cps")
                for kt in range(nkt):
                    off = max(kt - 4 * g, 0) * P
                    nc.tensor.matmul(
                        cps[:, off:512],
                        lhsT=v_sb[:, kt, h * P:(h + 1) * P],
                        rhs=ptg[:, kt, off:512],
                        start=(kt == 0), stop=(kt == nkt - 1),
                        skip_group_check=True,
                    )
                nc.vector.tensor_tensor(
                    out=ctxT_sb[:, h, g * 512:(g + 1) * 512],
                    in0=cps, in1=recip_bc, op=ALU.mult,
                )

            def emit_qkdef_units(h):
                """Deferred (wq, win2) projection pass for head h: dense PE
                filler for the early attention steps (which have no out-proj
                tiles yet).  Yields every 4 matmuls.  PSUM comes from the o
                ring so this never serializes behind the den->recip chain."""
                ps = o_pool.tile([P, 512], F32, tag="o", name="qkdps")
                for i in range(NI):
                    nc.tensor.matmul(
                        ps,
                        lhsT=wq_sb[:, i, h * P:(h + 1) * P],
                        rhs=xt2_sb[:, i, :],
                        start=(i == 0), stop=(i == NI - 1),
                        skip_group_check=True,
                    )
                    if i % 4 == 3 and i < NI - 1:
                        yield
                nc.vector.tensor_copy(out=qt_sb[:, h, 1024:1536], in_=ps)
                yield

            op_queue = []  # ready out-projection (nt, jc) tiles
            op_count = [0]

            def emit_op_tile(allow_act=False):
                nt, jc = op_queue.pop(0)
                ops = o_pool.tile([P, 512], F32, tag="o", name="ops")
                for hh in range(HPC):
                    nc.tensor.matmul(
                        ops,
                        lhsT=ctxT_sb[:, hh, nt * P:(nt + 1) * P],
                        rhs=wo_sb[:, hh, jc * 512:(jc + 1) * 512],
                        start=(hh == 0), stop=(hh == HPC - 1),
                    )
                osb = out_pool.tile([P, 512], BF16, tag="osb", name="osb")
                op_count[0] += 1
                if allow_act and op_count[0] % 2 == 0:
                    nc.scalar.copy(out=osb, in_=ops)
                else:
                    nc.vector.tensor_copy(out=osb, in_=ops)
                nc.sync.dma_start(
                    out_d[nt * P:(nt + 1) * P, jc * 512:(jc + 1) * 512], osb)

            next_sc = 1  # (3,0) was woven into projection
            trees = {0: emit_tree(3, 0, wv_ptg0)}
            defgens = []  # deferred (wq, win2) pass generators
            for idx, (g, h) in enumerate(steps):
                if idx < HPC:
                    defgens.append(emit_qkdef_units(idx))
                # emit scores up to LAG steps ahead, interleaved below
                gens = []
                while next_sc <= min(idx + LAG, len(steps) - 1):
                    g2, h2 = steps[next_sc]
                    ptg2 = att_pool.tile([P, NT, 512], BF16, tag="ptg",
                                         name=f"ptg_{g2}_{h2}")
                    memset_diag_holes(g2, ptg2)
                    ptgs[(g2, h2)] = ptg2
                    gens.append(emit_scores_units(g2, h2, ptg2, s_pool))
                    next_sc += 1

                def pump(n=1):
                    while n > 0 and gens:
                        try:
                            next(gens[0])
                            n -= 1
                        except StopIteration:
                            gens.pop(0)

                budget = 4 if idx < 8 else 7
                ptg = ptgs.pop((g, h))
                # ready score matmuls first so the PE never waits on the
                # denominator tree finishing on DVE
                pump(2)
                # denominator tree for the NEXT step (its P^T finished during
                # the previous step's pumps); runs on DVE under this step's
                # PE work.  At idx 0 the (3,1) P^T isn't done yet - deferred.
                if idx > 0 and idx + 1 < len(steps):
                    g2, h2 = steps[idx + 1]
                    trees[idx + 1] = emit_tree(g2, h2, ptgs[(g2, h2)])
                if idx == 0 and defgens:
                    # dense PE cover while tree(0) drains on DVE behind the
                    # final QK-B cast
                    for _ in range(2):
                        try:
                            next(defgens[0])
                        except StopIteration:
                            defgens.pop(0)
                recip_bc = emit_den(trees.pop(idx))
                emit_ctx(g, h, ptg, recip_bc)
                # interleave remaining score units with dense filler:
                # deferred (wq, win2) projection chunks, then out-proj tiles
                for _ in range(10):
                    pump(1)
                    if defgens:
                        try:
                            next(defgens[0])
                        except StopIteration:
                            defgens.pop(0)
                    elif budget > 0 and op_queue:
                        emit_op_tile(allow_act=(idx >= 6))
                        budget -= 1
                pump(100)
                while defgens:  # finish any deferred pass inside this step
                    try:
                        next(defgens[0])
                    except StopIteration:
                        defgens.pop(0)
                while budget > 0 and op_queue:
                    emit_op_tile(allow_act=(idx >= 6))
                    budget -= 1
                if idx == 0:
                    trees[1] = emit_tree(*steps[1], ptgs[steps[1]])
                if h == HPC - 1:
                    op_queue.extend(
                        (nt, jc) for nt in range(4 * g, 4 * g + 4)
                        for jc in range(NJ))
            while op_queue:
                emit_op_tile(allow_act=True)


def build_module():
    """Build and compile the per-core Bass module (SPMD: same program, 8 cores)."""
    nc = bacc.Bacc("TRN2", target_bir_lowering=False, debug=False,
                   num_devices=N_CORES)
    xt_d = nc.dram_tensor("xt", [D_IN, N_SEQ], BF16, kind="ExternalInput").ap()
    wq_d = nc.dram_tensor("wq", [D_IN, DC], BF16, kind="ExternalInput").ap()
    wk_d = nc.dram_tensor("wk", [D_IN, DC], BF16, kind="ExternalInput").ap()
    wv_d = nc.dram_tensor("wv", [D_IN, DC], BF16, kind="ExternalInput").ap()
    wo_d = nc.dram_tensor("wo", [DC, D_IN], BF16, kind="ExternalInput").ap()
    out_d = nc.dram_tensor("out", [N_SEQ, D_IN], BF16, kind="ExternalOutput").ap()
    with tile.TileContext(nc) as tc:
        _build_body(tc, xt_d, wq_d, wk_d, wv_d, wo_d, out_d)
    nc.compile()
    return nc


def make_in_maps(x, W_qkv, W_out):
    """Host-side sharding: per-core input dict, bf16 cast + pre-transposed x."""
    bf = ml_dtypes.bfloat16
    in_maps = []
    for c in range(N_CORES):
        b, g = divmod(c, 4)
        cs = slice(DC * g, DC * (g + 1))
        in_maps.append({
            "xt": np.ascontiguousarray(x[b].T).astype(bf),
            "wq": np.ascontiguousarray(W_qkv[:, 0 * D_IN:1 * D_IN][:, cs]).astype(bf),
            "wk": np.ascontiguousarray(W_qkv[:, 1 * D_IN:2 * D_IN][:, cs]).astype(bf),
            "wv": np.ascontiguousarray(W_qkv[:, 2 * D_IN:3 * D_IN][:, cs]).astype(bf),
            "wo": np.ascontiguousarray(W_out[cs, :]).astype(bf),
        })
    return in_maps


_NC_CACHE = {}


def get_module():
    if "nc" not in _NC_CACHE:
        _NC_CACHE["nc"] = build_module()
    return _NC_CACHE["nc"]


def run(x, W_qkv, W_out, b_out, trace=False, **trace_kwargs):
    nc = get_module()
    in_maps = make_in_maps(x, W_qkv, W_out)
    res = run_bass_kernel_spmd(nc, in_maps, core_ids=list(range(N_CORES)),
                               trace=trace, **trace_kwargs)
    parts = np.stack([np.asarray(res.results[c]["out"], dtype=np.float32)
                      for c in range(N_CORES)])
    parts = parts.reshape(2, 4, N_SEQ, D_IN)
    out = parts.sum(axis=1, dtype=np.float64).astype(np.float32)
    out += b_out.astype(np.float32)
    return out, res


def kernel(x, W_qkv, W_out, b_out):
    out, _ = run(np.asarray(x), np.asarray(W_qkv), np.asarray(W_out),
                 np.asarray(b_out))
    return out

